# revision 7
# baseline (speedup 1.0000x reference)
"""Trainium2 Bass kernel for nn_Attention_6519760355548 (ragged bag attention).

Math (per reference):
  seg = bag id per sentence (from scope offsets)
  logit[n,l] = x[n] . attention_weight[q[n,l]]
  w = segment_softmax(logit)             (per bag, per layer)
  bag[b,l,:] = sum_{n in bag b} w[n,l] * x[n,:]
  outputs: (bag.transpose(1,0,2), bag.reshape(B,3D), bag.reshape(B,3D)@R.T+bias)

Device strategy (8 cores, SPMD):
  - 512 bags per core, 4 PSUM blocks of 128 bags each.
  - Block sentences padded to T_B tiles of 128. Per tile:
      S = x_tile @ A.T  via 6 chunk matmuls on pre-transposed x (host supplies xT)
      W = exp(S)        (no max subtraction: logits are O(5), exp is safe)
      w[n,l] = W[n, q[n,l]]  via one-hot compare + fused multiply-reduce
      Mw_l[s,b] = (iota128[b]==segl[s]) * w_l[s]    (weighted membership)
      bagU_l[b, 0:769] += Mw_l.T @ [x_tile | 1]     (accumulate over tiles in PSUM)
  - bagU includes the softmax denominator in column 768 (ones column).
  - Host divides by the denominator, reshapes, and does the tiny probs matmul.
"""

import os
import sys

import numpy as np

if "/opt/trn_rl_repo" not in sys.path:
    sys.path.insert(0, "/opt/trn_rl_repo")

N_SENT = 65536
N_BAGS = 4096
D = 768
FLAT_C = 53
GLOBAL_C = 95
GC2 = 96        # class dim padded to even for fp32r matmul ISA
XW = 772        # x row: 768 x + ones col + 3 pad
NCORES = 8
NBLK = 4          # bag blocks per core (128 bags each; 8*4*128 = 4096)
BPB = 128         # bags per block
P = 128

# dtype knobs (iterate on these for perf; fp32/f32r is the accuracy baseline)
USE_BF16_LOGITS = os.environ.get("KB_BF16_LOGITS", "0") == "1"

_BUILD_CACHE = {}


def _build(T_B):
    import concourse.bacc as bacc
    import concourse.mybir as mybir
    import concourse.tile as tile
    from contextlib import ExitStack

    f32 = mybir.dt.float32
    f32r = mybir.dt.float32r
    bf16 = mybir.dt.bfloat16
    # fp32 matmul is 4 cyc/col; float32r streams 1 cyc/col at N>=256.
    # walrus requires every fp32r matmul operand to be produced as fp32r,
    # so the whole dataflow feeding matmuls is declared fp32r.
    ldt = bf16 if USE_BF16_LOGITS else f32r
    ie = mybir.AluOpType.is_equal
    mul = mybir.AluOpType.mult
    add = mybir.AluOpType.add
    Exp = mybir.ActivationFunctionType.Exp
    Copy = mybir.ActivationFunctionType.Copy

    S_pad = NBLK * T_B * P
    J = NBLK * T_B

    nc = bacc.Bacc(None, target_bir_lowering=False)
    xp = nc.dram_tensor("xp", [S_pad, XW], f32r, kind="ExternalInput")
    xptl = nc.dram_tensor("xptl", [J, P, 6 * P], ldt, kind="ExternalInput")
    smalls = nc.dram_tensor("smalls", [NBLK, P, T_B * 4], f32, kind="ExternalInput")
    atw = nc.dram_tensor("atw", [P, 6 * GC2], ldt, kind="ExternalInput")
    iotas = nc.dram_tensor("iotas", [P, GC2 + P], f32, kind="ExternalInput")
    bago = nc.dram_tensor("bago", [NBLK, P, 3 * (D + 1)], f32, kind="ExternalOutput")

    with ExitStack() as ctx:
        tc = ctx.enter_context(tile.TileContext(nc))
        const = ctx.enter_context(tc.tile_pool(name="const", bufs=1))
        xpool = ctx.enter_context(tc.tile_pool(name="x", bufs=4))
        xtpool = ctx.enter_context(tc.tile_pool(name="xt", bufs=4))
        spool = ctx.enter_context(tc.tile_pool(name="small", bufs=2))
        wpool = ctx.enter_context(tc.tile_pool(name="w", bufs=3))
        gpool = ctx.enter_context(tc.tile_pool(name="g", bufs=3))
        mpool = ctx.enter_context(tc.tile_pool(name="m", bufs=3))
        opool = ctx.enter_context(tc.tile_pool(name="o", bufs=2))
        ps_s = ctx.enter_context(tc.tile_pool(name="ps_s", bufs=2, space="PSUM"))
        ps_bag = ctx.enter_context(tc.tile_pool(name="ps_bag", bufs=1, space="PSUM"))

        at_t = const.tile([P, 6 * GC2], ldt)
        nc.sync.dma_start(at_t[:], atw[:, :])
        io_t = const.tile([P, GC2 + P], f32)
        nc.sync.dma_start(io_t[:], iotas[:, :])
        iota95 = io_t[:, :GC2]
        iota128 = io_t[:, GC2:]

        for p in range(NBLK):
            sm = spool.tile([P, T_B * 4], f32)
            nc.sync.dma_start(sm[:], smalls[p])
            # 3 layers x 1024 cols (769 used) -> 6 PSUM banks, bank-aligned slices
            bag_ps = ps_bag.tile([P, 3 * 1024], f32)

            for t in range(T_B):
                j = p * T_B + t
                x1 = xpool.tile([P, XW], f32r)
                nc.sync.dma_start(x1[:], xp[j * P:(j + 1) * P, :])
                xt6 = xtpool.tile([P, 6 * P], ldt)
                nc.sync.dma_start(xt6[:], xptl[j])

                # logits: S[n,c] = sum_d x[n,d] A[c,d], 6 chunks over d
                s_ps = ps_s.tile([P, GC2], f32)
                for c in range(6):
                    nc.tensor.matmul(
                        s_ps[:],
                        lhsT=xt6[:, c * P:(c + 1) * P],
                        rhs=at_t[:, c * GC2:(c + 1) * GC2],
                        start=(c == 0),
                        stop=(c == 5),
                    )

                w_t = wpool.tile([P, GC2], f32)
                nc.scalar.activation(w_t[:], s_ps[:], Exp)

                # w3[n,l] = W[n, q[n,l]] via one-hot mask, multiply, then a
                # copy-accumulate reduce on ScalarE (tensor_tensor_reduce
                # faults at runtime on this hw path)
                w3 = wpool.tile([P, 4], f32)
                for l in range(3):
                    oh = gpool.tile([P, GC2], f32, tag="oh")
                    nc.vector.tensor_scalar(
                        oh[:], iota95, sm[:, 4 * t + 1 + l:4 * t + 2 + l], None, ie
                    )
                    scr = gpool.tile([P, GC2], f32, tag="scr")
                    nc.vector.tensor_tensor(out=scr[:], in0=oh[:], in1=w_t[:], op=mul)
                    scr2 = gpool.tile([P, GC2], f32, tag="scr2")
                    nc.scalar.activation(
                        scr2[:], scr[:], Copy, accum_out=w3[:, l:l + 1]
                    )

                # weighted membership: Mw_l[s,b] = (iota128[b]==segl[s]) * w_l[s]
                mw = mpool.tile([P, 3 * P], f32r)
                for l in range(3):
                    nc.vector.tensor_scalar(
                        mw[:, l * P:(l + 1) * P],
                        iota128,
                        sm[:, 4 * t:4 * t + 1],
                        w3[:, l:l + 1],
                        ie,
                        mul,
                    )

                # aggregate: bagU_l += Mw_l.T @ [x|1]
                for l in range(3):
                    lhs = mw[:, l * P:(l + 1) * P]
                    nc.tensor.matmul(
                        bag_ps[:, l * 1024:l * 1024 + 512],
                        lhsT=lhs,
                        rhs=x1[:, 0:512],
                        start=(t == 0),
                        stop=(t == T_B - 1),
                    )
                    nc.tensor.matmul(
                        bag_ps[:, l * 1024 + 512:l * 1024 + 770],
                        lhsT=lhs,
                        rhs=x1[:, 512:770],
                        start=(t == 0),
                        stop=(t == T_B - 1),
                    )

            bo = opool.tile([P, 3 * (D + 1)], f32)
            for l in range(3):
                nc.scalar.activation(
                    bo[:, l * (D + 1):(l + 1) * (D + 1)],
                    bag_ps[:, l * 1024:l * 1024 + D + 1],
                    Copy,
                )
            nc.sync.dma_start(bago[p], bo[:])

    nc.compile()
    return nc


def _prep_inputs(x, q, scope):
    seg = np.repeat(np.arange(N_BAGS, dtype=np.int64), np.diff(scope))
    bounds = scope[np.arange(0, N_BAGS + 1, BPB)]
    blk_cnt = np.diff(bounds)  # 32 global blocks (core-major order)
    T_B = int(np.ceil(blk_cnt.max() / P))
    S_pad = NBLK * T_B * P
    J = NBLK * T_B

    ldt = np.float32  # bf16 handled via view cast below when enabled

    in_maps = []
    for c in range(NCORES):
        gidx = np.full((NBLK, T_B * P), -1, dtype=np.int64)
        for p in range(NBLK):
            g = c * NBLK + p
            s0, s1 = bounds[g], bounds[g + 1]
            gidx[p, : s1 - s0] = np.arange(s0, s1)
        gflat = gidx.reshape(-1)
        valid = gflat >= 0

        xpad = np.zeros((S_pad, XW), dtype=np.float32)
        xpad[valid, :D] = x[gflat[valid]]
        xpad[:, D] = 1.0

        # xT chunks per tile: xptl[j, dp, c*P+s] = x[sent(j,s), c*P+dp]
        xv = xpad[:, :D].reshape(J, P, 6, P)          # [j, s, c, dp]
        xptl = np.ascontiguousarray(xv.transpose(0, 3, 2, 1)).reshape(J, P, 6 * P)
        if USE_BF16_LOGITS:
            import jax.numpy as jnp
            xptl = np.asarray(jnp.asarray(xptl, jnp.bfloat16))

        segl = np.full((NBLK, T_B * P), -1.0, dtype=np.float32)
        for p in range(NBLK):
            g = c * NBLK + p
            s0, s1 = bounds[g], bounds[g + 1]
            segl[p, : s1 - s0] = (seg[s0:s1] - g * BPB).astype(np.float32)
        qpad = np.zeros((S_pad, 3), dtype=np.float32)
        qpad[valid] = q[gflat[valid]].astype(np.float32)
        sm = np.zeros((NBLK, T_B, P, 4), dtype=np.float32)
        sm[:, :, :, 0] = segl.reshape(NBLK, T_B, P)
        sm[:, :, :, 1:] = qpad.reshape(NBLK, T_B, P, 3)
        smalls = np.ascontiguousarray(sm.transpose(0, 2, 1, 3)).reshape(
            NBLK, P, T_B * 4
        )

        in_maps.append({"xp": xpad, "xptl": xptl, "smalls": smalls})
    return in_maps, T_B


_LAST_RESULT = None  # test.py reads exec_time_ns off this


def kernel(**inputs):
    global _LAST_RESULT
    x = np.asarray(inputs["x"], dtype=np.float32)
    q = np.asarray(inputs["attention_query"]).astype(np.int64)
    scope = np.asarray(inputs["scope"]).astype(np.int64)
    aw = np.asarray(inputs["attention_weight"], dtype=np.float32)
    rw = np.asarray(inputs["relation_weight"], dtype=np.float32)
    bias = np.asarray(inputs["bias"], dtype=np.float32)

    in_maps, T_B = _prep_inputs(x, q, scope)

    # replicated constants
    atp = np.zeros((6 * P, GC2), dtype=np.float32)
    atp[:, :GLOBAL_C] = aw.T
    at_chunks = np.ascontiguousarray(
        atp.reshape(6, P, GC2).transpose(1, 0, 2)
    ).reshape(P, 6 * GC2)
    if USE_BF16_LOGITS:
        import jax.numpy as jnp
        at_chunks = np.asarray(jnp.asarray(at_chunks, jnp.bfloat16))
    iotas = np.concatenate(
        [
            np.tile(np.arange(GC2, dtype=np.float32), (P, 1)),
            np.tile(np.arange(P, dtype=np.float32), (P, 1)),
        ],
        axis=1,
    )
    for m in in_maps:
        m["atw"] = at_chunks
        m["iotas"] = iotas

    key = (T_B, USE_BF16_LOGITS)
    if key not in _BUILD_CACHE:
        _BUILD_CACHE[key] = _build(T_B)
    nc = _BUILD_CACHE[key]

    from concourse.bass_utils import run_bass_kernel_spmd

    res = run_bass_kernel_spmd(nc, in_maps, core_ids=list(range(NCORES)))
    _LAST_RESULT = res

    bagu = np.stack([res.results[c]["bago"] for c in range(NCORES)])
    bagu = bagu.reshape(N_BAGS, 3, D + 1)
    denom = bagu[:, :, D:D + 1]
    bag = bagu[:, :, :D] / denom
    logits_total = np.ascontiguousarray(bag.reshape(N_BAGS, 3 * D), dtype=np.float32)
    stack = np.ascontiguousarray(bag.transpose(1, 0, 2), dtype=np.float32)
    probs = (logits_total @ rw.T + bias).astype(np.float32)
    return (stack, logits_total, probs)
